# revision 6
# baseline (speedup 1.0000x reference)
"""GraphSAGE (2-layer SAGEConv + log_softmax) fused into ONE kernel on 8
Trainium2 NeuronCores.

Sharding: nodes partitioned contiguously across 8 cores (6250 each, padded to
6400 = 50 tiles of 128 slots); within a core, nodes dealt round-robin by
degree so per-tile edge counts balance.

Math (exact up to fp reassociation / bf16 rounding):
  l1: aggT = segsum_e(x_bf16[src_e] ^T one-hot(dst_e))      (PE matmuls)
      mean^T = aggT * (1/deg)                                (DVE, at PSUM copy)
      h^T = relu(Wl1^T mean^T + Wr1^T x^T + b1)              (PE + Act)
  l2: z^T|r^T = (Wl2|Wr2)^T h^T  applied BEFORE aggregation (valid since
      segment-mean commutes with the linear map)
      z rows AllGather'ed across cores (collective), then
      out = log_softmax(segmean_e(z[src_e]) + r + b2)

Edge gathers use SWDGE dma_gather (int16 indices, 256B bf16 rows). int16 tops
out at 32767, so gather sources are split into low/high halves and each
tile's edge list is sorted into (low | high) segments, each zero-padded to a
multiple of 128 (pads gather row 0; their one-hot column is 0 so they never
contribute). Segment sizes are unified across cores (max) so all 8 cores
share one compiled kernel. One-hots are built on DVE via is_equal(dstv, iota).

The builder takes `reps`: the body is unrolled N times inside one NEFF so
device time per rep can be measured as a launch-count slope, immune to the
large noisy axon-tunnel launch overhead.
"""
import numpy as np
import ml_dtypes

import concourse.bass as bass
import concourse.bacc as bacc
import concourse.mybir as mybir
import concourse.tile as tile
from concourse import bass_utils

F32 = mybir.dt.float32
BF16 = mybir.dt.bfloat16
I16 = mybir.dt.int16
AF = mybir.ActivationFunctionType
OP = mybir.AluOpType
P = 128

N_NODES = 50000
N_EDGES = 400000
IN_CH = 128
HID = 1024
OUT_CH = 47
NCORES = 8
NPC = N_NODES // NCORES          # 6250
NTILES = 50
SLOTS = NTILES * P               # 6400
HB = HID // P                    # 8
XLO = 32768                      # int16 index range split for x
ZSPLIT = (NCORES * SLOTS) // 2   # 25600, split for z_all


def build_fused(seg1, seg2, reps=1):
    """seg1/seg2: per tile (offA_cols, nA, offB_cols, nB, off_chunks),
    identical across cores."""
    c1max = max((nA + nB) // P for _, nA, _, nB, _ in seg1)
    c2max = max((nA + nB) // P for _, nA, _, nB, _ in seg2)
    W1 = sum((nA + nB) // 16 for _, nA, _, nB, _ in seg1)
    W2c = sum((nA + nB) // 16 for _, nA, _, nB, _ in seg2)
    T1 = sum((nA + nB) // P for _, nA, _, nB, _ in seg1)
    T2 = sum((nA + nB) // P for _, nA, _, nB, _ in seg2)

    nc = bacc.Bacc("TRN2", target_bir_lowering=False, debug=False,
                   enable_asserts=False, num_devices=NCORES)
    x_lo = nc.dram_tensor("x_lo", [XLO, P], BF16, kind="ExternalInput").ap()
    x_hi = nc.dram_tensor("x_hi", [N_NODES - XLO, P], BF16, kind="ExternalInput").ap()
    xT = nc.dram_tensor("xT", [P, SLOTS], BF16, kind="ExternalInput").ap()
    idx1 = nc.dram_tensor("idx1", [P, W1], I16, kind="ExternalInput").ap()
    idx2 = nc.dram_tensor("idx2", [P, W2c], I16, kind="ExternalInput").ap()
    dstv1 = nc.dram_tensor("dstv1", [P, T1], BF16, kind="ExternalInput").ap()
    dstv2 = nc.dram_tensor("dstv2", [P, T2], BF16, kind="ExternalInput").ap()
    winv1 = nc.dram_tensor("winv1", [P, SLOTS], BF16, kind="ExternalInput").ap()
    winv2 = nc.dram_tensor("winv2", [P, NTILES], F32, kind="ExternalInput").ap()
    Wl1 = nc.dram_tensor("Wl1", [P, HID], BF16, kind="ExternalInput").ap()
    Wr1 = nc.dram_tensor("Wr1", [P, HID], BF16, kind="ExternalInput").ap()
    W2 = nc.dram_tensor("W2", [P, HB, 2 * OUT_CH], BF16, kind="ExternalInput").ap()
    b1c = nc.dram_tensor("b1c", [P, HB], F32, kind="ExternalInput").ap()
    b2rep = nc.dram_tensor("b2rep", [P, OUT_CH], F32, kind="ExternalInput").ap()
    iota = nc.dram_tensor("iota", [P, P], BF16, kind="ExternalInput").ap()
    identf = nc.dram_tensor("identf", [P, P], F32, kind="ExternalInput").ap()
    out = nc.dram_tensor("out", [SLOTS, OUT_CH], F32, kind="ExternalOutput").ap()

    with tile.TileContext(nc) as tc:
        with (
            tc.tile_pool(name="const", bufs=1) as cp,
            tc.tile_pool(name="work", bufs=3) as wp,
            tc.tile_pool(name="dram", bufs=1, space="DRAM") as dp,
            tc.tile_pool(name="ps_mag", bufs=2, space="PSUM") as psm,
            tc.tile_pool(name="ps_h", bufs=2, space="PSUM") as psh,
            tc.tile_pool(name="ps_o", bufs=2, space="PSUM") as pso_p,
            tc.tile_pool(name="ps_zr", bufs=1, space="PSUM") as psz,
            tc.tile_pool(name="ps_t", bufs=1, space="PSUM") as pst_p,
        ):
            # ---- constants (loaded once; excluded from per-rep marginal) ----
            idx1_sb = cp.tile([P, W1], I16)
            nc.sync.dma_start(out=idx1_sb[:], in_=idx1)
            idx2_sb = cp.tile([P, W2c], I16)
            nc.sync.dma_start(out=idx2_sb[:], in_=idx2)
            dstv1_sb = cp.tile([P, T1], BF16)
            nc.sync.dma_start(out=dstv1_sb[:], in_=dstv1)
            dstv2_sb = cp.tile([P, T2], BF16)
            nc.sync.dma_start(out=dstv2_sb[:], in_=dstv2)
            winv1_sb = cp.tile([P, SLOTS], BF16)
            nc.sync.dma_start(out=winv1_sb[:], in_=winv1)
            winv2_sb = cp.tile([P, NTILES], F32)
            nc.sync.dma_start(out=winv2_sb[:], in_=winv2)
            xT_sb = cp.tile([P, SLOTS], BF16)
            nc.sync.dma_start(out=xT_sb[:], in_=xT)
            wl1_sb = cp.tile([P, HID], BF16)
            nc.sync.dma_start(out=wl1_sb[:], in_=Wl1)
            wr1_sb = cp.tile([P, HID], BF16)
            nc.sync.dma_start(out=wr1_sb[:], in_=Wr1)
            w2_sb = cp.tile([P, HB, 2 * OUT_CH], BF16)
            nc.sync.dma_start(out=w2_sb[:], in_=W2)
            b1_sb = cp.tile([P, HB], F32)
            nc.sync.dma_start(out=b1_sb[:], in_=b1c)
            b2_sb = cp.tile([P, OUT_CH], F32)
            nc.sync.dma_start(out=b2_sb[:], in_=b2rep)
            iota_sb = cp.tile([P, P], BF16)
            nc.sync.dma_start(out=iota_sb[:], in_=iota)
            idf_sb = cp.tile([P, P], F32)
            nc.sync.dma_start(out=idf_sb[:], in_=identf)

            z_sb = cp.tile([P, NTILES, P], BF16)      # z rows staged (pad cols)
            nc.vector.memset(z_sb[:], 0.0)            # zero pad cols once
            r_sb = cp.tile([P, NTILES, OUT_CH], F32)
            t_st = cp.tile([P, NTILES, OUT_CH], F32)
            nmax_st = cp.tile([P, NTILES], F32)
            sume_st = cp.tile([P, NTILES], F32)
            lse_st = cp.tile([P, NTILES], F32)
            out_st = cp.tile([P, NTILES, OUT_CH], F32)

            z_my = dp.tile([SLOTS, P], BF16)
            z_all = dp.tile([NCORES * SLOTS, P], BF16)

            for _rep in range(reps):
                # ================= layer 1 =================
                for t in range(NTILES):
                    offA, nA, offB, nB, offc = seg1[t]
                    cA, cB = nA // P, nB // P
                    c1 = cA + cB
                    m1 = wp.tile([P, c1max, P], BF16, tag="m1")
                    if nA:
                        nc.gpsimd.dma_gather(
                            out_ap=m1[:, 0:cA, :], in_ap=x_lo,
                            idxs_ap=idx1_sb[:, offA:offA + nA // 16],
                            num_idxs=nA, num_idxs_reg=nA, elem_size=P)
                    if nB:
                        nc.gpsimd.dma_gather(
                            out_ap=m1[:, cA:c1, :], in_ap=x_hi,
                            idxs_ap=idx1_sb[:, offB:offB + nB // 16],
                            num_idxs=nB, num_idxs_reg=nB, elem_size=P)
                    oh = wp.tile([P, c1max, P], BF16, tag="oh1")
                    nc.vector.tensor_tensor(
                        out=oh[:, 0:c1, :],
                        in0=dstv1_sb[:, offc:offc + c1].rearrange(
                            "p (c d) -> p c d", d=1).to_broadcast([P, c1, P]),
                        in1=iota_sb[:].rearrange("p (c d) -> p c d", c=1)
                            .to_broadcast([P, c1, P]),
                        op=OP.is_equal)
                    ps_mag = psm.tile([P, P], F32, space="PSUM", tag="psmag")
                    for c in range(c1):
                        nc.tensor.matmul(out=ps_mag[:], lhsT=m1[:, c, :],
                                         rhs=oh[:, c, :],
                                         start=(c == 0), stop=(c == c1 - 1))
                    mag_sb = wp.tile([P, P], BF16, tag="mag")
                    nc.vector.tensor_tensor(
                        out=mag_sb[:], in0=ps_mag[:],
                        in1=winv1_sb[:, t * P:(t + 1) * P], op=OP.mult)
                    ht = wp.tile([P, HB, P], BF16, tag="ht")
                    for j in range(HB):
                        ps_h = psh.tile([P, P], F32, space="PSUM", tag="psh")
                        nc.tensor.matmul(out=ps_h[:], lhsT=wl1_sb[:, j * P:(j + 1) * P],
                                         rhs=mag_sb[:], start=True, stop=False)
                        nc.tensor.matmul(out=ps_h[:], lhsT=wr1_sb[:, j * P:(j + 1) * P],
                                         rhs=xT_sb[:, t * P:(t + 1) * P],
                                         start=False, stop=True)
                        nc.scalar.activation(out=ht[:, j, :], in_=ps_h[:],
                                             func=AF.Relu, bias=b1_sb[:, j:j + 1],
                                             scale=1.0)
                    ps_zr = psz.tile([2 * OUT_CH, P], F32, space="PSUM", tag="pszr")
                    for j in range(HB):
                        nc.tensor.matmul(out=ps_zr[:], lhsT=w2_sb[:, j, :],
                                         rhs=ht[:, j, :],
                                         start=(j == 0), stop=(j == HB - 1))
                    zr_sb = wp.tile([2 * OUT_CH, P], F32, tag="zr")
                    nc.vector.tensor_copy(out=zr_sb[:], in_=ps_zr[:])
                    ps_t = pst_p.tile([P, 2 * OUT_CH], F32, space="PSUM", tag="pst")
                    nc.tensor.transpose(out=ps_t[:], in_=zr_sb[:],
                                        identity=idf_sb[0:2 * OUT_CH, 0:2 * OUT_CH])
                    nc.vector.tensor_copy(out=z_sb[:, t, 0:OUT_CH],
                                          in_=ps_t[:, 0:OUT_CH])
                    nc.vector.tensor_tensor(out=r_sb[:, t, :],
                                            in0=ps_t[:, OUT_CH:2 * OUT_CH],
                                            in1=b2_sb[:], op=OP.add)

                # ============ exchange z across cores ============
                nc.sync.dma_start(
                    out=z_my[:].rearrange("(t p) c -> p t c", p=P), in_=z_sb[:])
                nc.gpsimd.collective_compute(
                    "AllGather", OP.bypass,
                    replica_groups=[list(range(NCORES))],
                    ins=[z_my[:].opt()], outs=[z_all[:].opt()])

                # ================= layer 2 =================
                for t in range(NTILES):
                    offA, nA, offB, nB, offc = seg2[t]
                    cA, cB = nA // P, nB // P
                    c2 = cA + cB
                    m2 = wp.tile([P, c2max, P], BF16, tag="m2")
                    if nA:
                        nc.gpsimd.dma_gather(
                            out_ap=m2[:, 0:cA, :], in_ap=z_all[0:ZSPLIT, :],
                            idxs_ap=idx2_sb[:, offA:offA + nA // 16],
                            num_idxs=nA, num_idxs_reg=nA, elem_size=P)
                    if nB:
                        nc.gpsimd.dma_gather(
                            out_ap=m2[:, cA:c2, :], in_ap=z_all[ZSPLIT:, :],
                            idxs_ap=idx2_sb[:, offB:offB + nB // 16],
                            num_idxs=nB, num_idxs_reg=nB, elem_size=P)
                    oh2 = wp.tile([P, c2max, P], BF16, tag="oh2")
                    nc.vector.tensor_tensor(
                        out=oh2[:, 0:c2, :],
                        in0=dstv2_sb[:, offc:offc + c2].rearrange(
                            "p (c d) -> p c d", d=1).to_broadcast([P, c2, P]),
                        in1=iota_sb[:].rearrange("p (c d) -> p c d", c=1)
                            .to_broadcast([P, c2, P]),
                        op=OP.is_equal)
                    ps_o = pso_p.tile([P, 64], F32, space="PSUM", tag="pso")
                    for c in range(c2):
                        nc.tensor.matmul(out=ps_o[:], lhsT=oh2[:, c, :],
                                         rhs=m2[:, c, 0:64],
                                         start=(c == 0), stop=(c == c2 - 1))
                    # t = agg*winv + (r+b2)
                    nc.vector.tensor_scalar(
                        out=t_st[:, t, :], in0=ps_o[:, 0:OUT_CH],
                        scalar1=winv2_sb[:, t:t + 1], scalar2=None, op0=OP.mult)
                    nc.vector.tensor_tensor(out=t_st[:, t, :], in0=t_st[:, t, :],
                                            in1=r_sb[:, t, :], op=OP.add)
                    rmax = wp.tile([P, 1], F32, tag="rmax")
                    nc.vector.tensor_reduce(out=rmax[:], in_=t_st[:, t, :],
                                            axis=mybir.AxisListType.X, op=OP.max)
                    nc.vector.tensor_scalar_mul(out=nmax_st[:, t:t + 1],
                                                in0=rmax[:], scalar1=-1.0)
                    e_sb = wp.tile([P, OUT_CH], F32, tag="esb")
                    nc.scalar.activation(out=e_sb[:], in_=t_st[:, t, :],
                                         func=AF.Exp, bias=nmax_st[:, t:t + 1],
                                         scale=1.0, accum_out=sume_st[:, t:t + 1])
                # ============ log-softmax finalize ============
                nc.scalar.activation(out=lse_st[:], in_=sume_st[:], func=AF.Ln)
                for t in range(NTILES):
                    nc.vector.tensor_scalar(
                        out=out_st[:, t, :], in0=t_st[:, t, :],
                        scalar1=nmax_st[:, t:t + 1], scalar2=lse_st[:, t:t + 1],
                        op0=OP.add, op1=OP.subtract)
                nc.sync.dma_start(
                    out=out.rearrange("(t p) c -> p t c", p=P), in_=out_st[:])
    nc.compile()
    return nc


# ---------------------------------------------------------------------------
# host-side preprocessing
# ---------------------------------------------------------------------------

def _wrap_idx(idx):
    """idx array (len multiple of 16) -> [128, len/16] int16 (k -> [k%16,
    k//16]), replicated to the 8 q7 groups."""
    n = len(idx)
    arr = np.zeros((16, n // 16), np.int16)
    arr[np.arange(n) % 16, np.arange(n) // 16] = idx
    return np.tile(arr, (8, 1))


def _prep(x, edge_index, Wl1, Wr1, b1, Wl2, Wr2, b2):
    src = edge_index[0].astype(np.int64)
    dst = edge_index[1].astype(np.int64)
    deg = np.bincount(dst, minlength=N_NODES)
    winv = 1.0 / np.maximum(deg, 1).astype(np.float32)

    # per-core slot assignment: deal nodes to tiles round-robin by degree
    slot_of = np.empty(N_NODES, np.int64)
    for c in range(NCORES):
        nids = np.arange(c * NPC, (c + 1) * NPC)
        order = nids[np.argsort(-deg[nids], kind="stable")]
        k = np.arange(NPC)
        slot_of[order] = (k % NTILES) * P + (k // NTILES)
    ecore = dst // NPC
    dslot = slot_of[dst]
    dtile = dslot // P
    dlane = dslot % P
    score = np.minimum(src // NPC, NCORES - 1)
    gsrc = score * SLOTS + slot_of[src]       # global z_all row of src

    # collect per (core, tile) lo/hi edge lists for both layers
    ed1 = [[None] * NTILES for _ in range(NCORES)]
    ed2 = [[None] * NTILES for _ in range(NCORES)]
    for c in range(NCORES):
        sel = np.nonzero(ecore == c)[0]
        for t in range(NTILES):
            es = sel[dtile[sel] == t]
            lo = es[src[es] < XLO]
            hi = es[src[es] >= XLO]
            ed1[c][t] = (lo[np.argsort(src[lo], kind="stable")],
                         hi[np.argsort(src[hi], kind="stable")])
            lo = es[gsrc[es] < ZSPLIT]
            hi = es[gsrc[es] >= ZSPLIT]
            ed2[c][t] = (lo[np.argsort(gsrc[lo], kind="stable")],
                         hi[np.argsort(gsrc[hi], kind="stable")])

    # unified segment sizes (max across cores, rounded to 128)
    def unify(ed):
        seg = []
        offw = offc = 0
        for t in range(NTILES):
            nA = max(len(ed[c][t][0]) for c in range(NCORES))
            nB = max(len(ed[c][t][1]) for c in range(NCORES))
            nA = ((nA + P - 1) // P) * P
            nB = ((nB + P - 1) // P) * P
            seg.append((offw, nA, offw + nA // 16, nB, offc))
            offw += (nA + nB) // 16
            offc += (nA + nB) // P
        return seg

    seg1 = unify(ed1)
    seg2 = unify(ed2)

    def pack(ed, seg, idx_of):
        """-> (idx [128, W], dstv [128, T]) for one core."""
        idxs, dvs = [], []
        for t in range(NTILES):
            _, nA, _, nB, _ = seg[t]
            lo, hi = ed[t]
            for es, n, rebase in ((lo, nA, 0), (hi, nB, 1)):
                iv = np.zeros(n, np.int16)
                dv = np.full(n, -1.0, np.float32)
                iv[:len(es)] = idx_of(es, rebase)
                dv[:len(es)] = dlane[es]
                idxs.append(iv)
                dvs.append(dv)
        idx = _wrap_idx(np.concatenate(idxs))
        dv = np.concatenate(dvs).reshape(-1, P).T.astype(ml_dtypes.bfloat16)
        return idx, np.ascontiguousarray(dv)

    x_lo = x[:XLO].astype(ml_dtypes.bfloat16)
    x_hi = x[XLO:].astype(ml_dtypes.bfloat16)
    iota = np.tile(np.arange(P, dtype=np.float32)[None, :], (P, 1))
    b1c = b1.reshape(HB, P).T.astype(np.float32).copy()
    W2h = np.ascontiguousarray(
        np.concatenate([Wl2, Wr2], axis=1).reshape(HB, P, 2 * OUT_CH)
        .transpose(1, 0, 2)).astype(ml_dtypes.bfloat16)

    in_maps = []
    for c in range(NCORES):
        i1, d1 = pack(ed1[c], seg1,
                      lambda es, hi: (src[es] - (XLO if hi else 0)).astype(np.int16))
        i2, d2 = pack(ed2[c], seg2,
                      lambda es, hi: (gsrc[es] - (ZSPLIT if hi else 0)).astype(np.int16))
        nids = np.arange(c * NPC, (c + 1) * NPC)
        xs = np.zeros((SLOTS, IN_CH), np.float32)
        xs[slot_of[nids]] = x[nids]
        winv_slots = np.zeros(SLOTS, np.float32)
        winv_slots[slot_of[nids]] = winv[nids]
        in_maps.append({
            "x_lo": x_lo, "x_hi": x_hi,
            "xT": np.ascontiguousarray(xs.T).astype(ml_dtypes.bfloat16),
            "idx1": i1, "idx2": i2, "dstv1": d1, "dstv2": d2,
            "winv1": np.broadcast_to(
                winv_slots.astype(ml_dtypes.bfloat16), (P, SLOTS)).copy(),
            "winv2": np.ascontiguousarray(
                winv_slots.reshape(NTILES, P).T).astype(np.float32),
            "Wl1": Wl1.astype(ml_dtypes.bfloat16),
            "Wr1": Wr1.astype(ml_dtypes.bfloat16),
            "W2": W2h, "b1c": b1c,
            "b2rep": np.broadcast_to(b2.astype(np.float32), (P, OUT_CH)).copy(),
            "iota": iota.astype(ml_dtypes.bfloat16),
            "identf": np.eye(P, dtype=np.float32),
        })
    return seg1, seg2, in_maps, slot_of


_cache = {}


def _get_nc(seg1, seg2, reps=1):
    key = (tuple(seg1), tuple(seg2), reps)
    if key not in _cache:
        _cache[key] = build_fused(seg1, seg2, reps=reps)
    return _cache[key]


def kernel(x, edge_index, Wl1, Wr1, b1, Wl2, Wr2, b2):
    x = np.asarray(x, np.float32)
    edge_index = np.asarray(edge_index)
    seg1, seg2, in_maps, slot_of = _prep(
        x, edge_index, np.asarray(Wl1, np.float32), np.asarray(Wr1, np.float32),
        np.asarray(b1, np.float32), np.asarray(Wl2, np.float32),
        np.asarray(Wr2, np.float32), np.asarray(b2, np.float32))
    nc = _get_nc(seg1, seg2, reps=1)
    res = bass_utils.run_bass_kernel_spmd(nc, in_maps, core_ids=list(range(NCORES)))
    out = np.empty((N_NODES, OUT_CH), np.float32)
    for c in range(NCORES):
        o = res.results[c]["out"]
        nids = np.arange(c * NPC, (c + 1) * NPC)
        out[nids] = o[slot_of[nids]]
    return out


# ---------------------------------------------------------------------------
# device-time measurement: launch-count slopes of 1-rep vs R-rep kernels.
# per-rep exec = (slope(R) - slope(1)) / (R - 1); tunnel/dispatch overhead
# cancels in the slope difference.
# ---------------------------------------------------------------------------

def _make_runner(nc, n_cores):
    import jax
    from jax.sharding import Mesh, PartitionSpec, NamedSharding
    from jax.experimental.shard_map import shard_map
    from concourse import bass2jax

    bass2jax.install_neuronx_cc_hook()
    pname = nc.partition_id_tensor.name if nc.partition_id_tensor else None
    in_names, out_names, out_avals = [], [], []
    for alloc in nc.m.functions[0].allocations:
        if not isinstance(alloc, mybir.MemoryLocationSet):
            continue
        name = alloc.memorylocations[0].name
        if alloc.kind == "ExternalInput":
            if name != pname:
                in_names.append(name)
        elif alloc.kind == "ExternalOutput":
            out_names.append(name)
            out_avals.append(jax.core.ShapedArray(
                tuple(alloc.tensor_shape), mybir.dt.np(alloc.dtype)))
    n_params = len(in_names)
    all_in = list(in_names) + list(out_names)
    if pname is not None:
        all_in.append(pname)

    def _body(*args):
        operands = list(args)
        if pname is not None:
            operands.append(bass2jax.partition_id_tensor())
        outs = bass2jax._bass_exec_p.bind(
            *operands, out_avals=tuple(out_avals), in_names=tuple(all_in),
            out_names=tuple(out_names), lowering_input_output_aliases=(),
            sim_require_finite=False, sim_require_nnan=False, nc=nc)
        return tuple(outs)

    devices = jax.devices()[:n_cores]
    mesh = Mesh(np.asarray(devices), ("core",))
    jitted = jax.jit(
        shard_map(_body, mesh=mesh,
                  in_specs=(PartitionSpec("core"),) * (n_params + len(out_names)),
                  out_specs=(PartitionSpec("core"),) * len(out_names),
                  check_rep=False),
        keep_unused=True)

    def prep(in_maps):
        concat = [np.concatenate([np.asarray(in_maps[c][n]) for c in range(n_cores)], 0)
                  for n in in_names]
        zeros = [np.zeros((n_cores * a.shape[0], *a.shape[1:]), a.dtype)
                 for a in out_avals]
        sh = NamedSharding(mesh, PartitionSpec("core"))
        return [jax.device_put(v, sh) for v in concat + zeros]

    return jitted, prep


def _slope(jitted, dev_in, ks=(1, 9, 17), sweeps=5):
    """Marginal wall time per pipelined call, via least squares over k."""
    import time as _time
    import jax
    jax.block_until_ready(jitted(*dev_in))
    samples = {k: [] for k in ks}
    for _ in range(sweeps):
        for k in ks:
            t0 = _time.perf_counter()
            for _i in range(k):
                o = jitted(*dev_in)
            jax.block_until_ready(o)
            samples[k].append(_time.perf_counter() - t0)
    xs = np.array(ks, np.float64)
    ys = np.array([np.median(samples[k]) for k in ks])
    A = np.stack([xs, np.ones_like(xs)], axis=1)
    slope, _ = np.linalg.lstsq(A, ys, rcond=None)[0]
    return float(slope)


def measure_exec_ns(inp, reps_hi=8):
    seg1, seg2, in_maps, _ = _prep(
        np.asarray(inp["x"], np.float32), np.asarray(inp["edge_index"]),
        np.asarray(inp["Wl1"], np.float32), np.asarray(inp["Wr1"], np.float32),
        np.asarray(inp["b1"], np.float32), np.asarray(inp["Wl2"], np.float32),
        np.asarray(inp["Wr2"], np.float32), np.asarray(inp["b2"], np.float32))
    nc1 = _get_nc(seg1, seg2, reps=1)
    ncR = _get_nc(seg1, seg2, reps=reps_hi)
    j1, prep1 = _make_runner(nc1, NCORES)
    d1 = prep1(in_maps)
    jR, prepR = _make_runner(ncR, NCORES)
    dR = prepR(in_maps)
    s1 = _slope(j1, d1)
    sR = _slope(jR, dR)
    per = (sR - s1) / (reps_hi - 1)
    print(f"  [slope-timing] slope(1)={s1*1e3:.3f} ms/call "
          f"slope({reps_hi})={sR*1e3:.3f} ms/call -> per-rep {per*1e6:.1f} us")
    return max(int(per * 1e9), 1)


# revision 22
# speedup vs baseline: 1.8118x; 1.8118x over previous
"""GraphSAGE (2-layer SAGEConv + log_softmax) fused into ONE kernel on 8
Trainium2 NeuronCores.

Sharding: nodes partitioned contiguously across 8 cores (6250 each, padded to
6400 = 50 tiles of 128 slots); within a core, nodes dealt round-robin by
degree so per-tile edge counts balance.

Math (exact up to fp reassociation / bf16 rounding):
  l1: aggT = segsum_e(x_bf16[src_e] ^T one-hot(dst_e))      (PE matmuls)
      mean^T = aggT * (1/deg)                                (DVE, at PSUM copy)
      h^T = relu(Wl1^T mean^T + Wr1^T x^T + b1)              (PE + Act)
  l2: z^T|r^T = (Wl2|Wr2)^T h^T  applied BEFORE aggregation (valid since
      segment-mean commutes with the linear map)
      z rows AllGather'ed across cores (collective), then
      out = log_softmax(segmean_e(z[src_e]) + r + b2)

Edge gathers use SWDGE dma_gather (int16 indices, 256B bf16 rows). int16 tops
out at 32767, so gather sources are split into low/high halves and each
tile's edge list is sorted into (low | high) segments, each zero-padded to a
multiple of 128 (pads gather row 0; their one-hot column is 0 so they never
contribute). Segment sizes are unified across cores (max) so all 8 cores
share one compiled kernel. One-hots are built on DVE via is_equal(dstv, iota).

The builder takes `reps`: the body is unrolled N times inside one NEFF so
device time per rep can be measured as a launch-count slope, immune to the
large noisy axon-tunnel launch overhead.
"""
import numpy as np
import ml_dtypes

import concourse.bass as bass
import concourse.bacc as bacc
import concourse.mybir as mybir
import concourse.tile as tile
from concourse import bass_utils

F32 = mybir.dt.float32
BF16 = mybir.dt.bfloat16
I16 = mybir.dt.int16
AF = mybir.ActivationFunctionType
OP = mybir.AluOpType
P = 128

N_NODES = 50000
N_EDGES = 400000
IN_CH = 128
HID = 1024
OUT_CH = 47
NCORES = 8
NPC = N_NODES // NCORES          # 6250
NTILES = 50
SLOTS = NTILES * P               # 6400
HB = HID // P                    # 8
XLO = 32768                      # int16 index range split for x
ZSPLIT = (NCORES * SLOTS) // 2   # 25600, split for z_all


def _chunk_klims(nA, nB, nA16, nB16):
    """Per chunk (lane-count to contract): unwritten pad lanes are excluded
    from the matmul so they are never read."""
    ks = []
    for c in range(nA // P):
        ks.append(min(P, nA16 - c * P))
    for c in range(nB // P):
        ks.append(min(P, nB16 - c * P))
    return ks


def build_fused(seg1, seg2, reps=1, no_ag=False, no_g1=False, no_g2=False,
                no_mm=False, no_dve=False):
    """seg1/seg2: per tile (offA_cols, nA, offB_cols, nB, off_chunks),
    identical across cores. no_* flags build timing-only bisect variants."""
    c1max = max((s[1] + s[3]) // P for s in seg1)
    c2max = max((s[1] + s[3]) // P for s in seg2)
    W1 = sum((s[1] + s[3]) // 16 for s in seg1)
    W2c = sum((s[1] + s[3]) // 16 for s in seg2)
    T1 = sum((s[1] + s[3]) // P for s in seg1)
    T2 = sum((s[1] + s[3]) // P for s in seg2)

    nc = bacc.Bacc("TRN2", target_bir_lowering=False, debug=False,
                   enable_asserts=False, num_devices=NCORES)
    x_lo = nc.dram_tensor("x_lo", [XLO, P], BF16, kind="ExternalInput").ap()
    x_hi = nc.dram_tensor("x_hi", [N_NODES - XLO, P], BF16, kind="ExternalInput").ap()
    xT = nc.dram_tensor("xT", [P, SLOTS], BF16, kind="ExternalInput").ap()
    idx1 = nc.dram_tensor("idx1", [P, W1], I16, kind="ExternalInput").ap()
    idx2 = nc.dram_tensor("idx2", [P, W2c], I16, kind="ExternalInput").ap()
    dstv1 = nc.dram_tensor("dstv1", [P, T1], BF16, kind="ExternalInput").ap()
    dstv2e = nc.dram_tensor("dstv2e", [P, T2], BF16, kind="ExternalInput").ap()
    dstv2o = nc.dram_tensor("dstv2o", [P, T2], BF16, kind="ExternalInput").ap()
    winv1 = nc.dram_tensor("winv1", [P, SLOTS], BF16, kind="ExternalInput").ap()
    winv2 = nc.dram_tensor("winv2", [P, NTILES], F32, kind="ExternalInput").ap()
    Wl1 = nc.dram_tensor("Wl1", [P, HID], BF16, kind="ExternalInput").ap()
    Wr1 = nc.dram_tensor("Wr1", [P, HID], BF16, kind="ExternalInput").ap()
    W2 = nc.dram_tensor("W2", [P, HB, 2 * OUT_CH], BF16, kind="ExternalInput").ap()
    b1c = nc.dram_tensor("b1c", [P, HB], F32, kind="ExternalInput").ap()
    b2rep = nc.dram_tensor("b2rep", [P, OUT_CH], F32, kind="ExternalInput").ap()
    iota = nc.dram_tensor("iota", [P, P], BF16, kind="ExternalInput").ap()
    identf = nc.dram_tensor("identf", [P, P], F32, kind="ExternalInput").ap()
    out = nc.dram_tensor("out", [SLOTS, OUT_CH], F32, kind="ExternalOutput").ap()

    with tile.TileContext(nc) as tc:
        with (
            tc.tile_pool(name="const", bufs=1) as cp,
            tc.tile_pool(name="work", bufs=3) as wp,
            tc.tile_pool(name="dram", bufs=1, space="DRAM") as dp,
            tc.tile_pool(name="ps_mag", bufs=2, space="PSUM") as psm,
            tc.tile_pool(name="ps_h", bufs=2, space="PSUM") as psh,
            tc.tile_pool(name="ps_o", bufs=2, space="PSUM") as pso_p,
            tc.tile_pool(name="ps_zr", bufs=1, space="PSUM") as psz,
            tc.tile_pool(name="ps_t", bufs=1, space="PSUM") as pst_p,
        ):
            # ---- constants (loaded once; excluded from per-rep marginal) ----
            idx1_sb = cp.tile([P, W1], I16)
            nc.sync.dma_start(out=idx1_sb[:], in_=idx1)
            idx2_sb = cp.tile([P, W2c], I16)
            nc.sync.dma_start(out=idx2_sb[:], in_=idx2)
            dstv1_sb = cp.tile([P, T1], BF16)
            nc.sync.dma_start(out=dstv1_sb[:], in_=dstv1)
            dstv2e_sb = cp.tile([P, T2], BF16)
            nc.sync.dma_start(out=dstv2e_sb[:], in_=dstv2e)
            dstv2o_sb = cp.tile([P, T2], BF16)
            nc.sync.dma_start(out=dstv2o_sb[:], in_=dstv2o)
            winv1_sb = cp.tile([P, SLOTS], BF16)
            nc.sync.dma_start(out=winv1_sb[:], in_=winv1)
            winv2_sb = cp.tile([P, NTILES], F32)
            nc.sync.dma_start(out=winv2_sb[:], in_=winv2)
            xT_sb = cp.tile([P, SLOTS], BF16)
            nc.sync.dma_start(out=xT_sb[:], in_=xT)
            wl1_sb = cp.tile([P, HID], BF16)
            nc.sync.dma_start(out=wl1_sb[:], in_=Wl1)
            wr1_sb = cp.tile([P, HID], BF16)
            nc.sync.dma_start(out=wr1_sb[:], in_=Wr1)
            w2_sb = cp.tile([P, HB, 2 * OUT_CH], BF16)
            nc.sync.dma_start(out=w2_sb[:], in_=W2)
            b1_sb = cp.tile([P, HB], F32)
            nc.sync.dma_start(out=b1_sb[:], in_=b1c)
            b2_sb = cp.tile([P, OUT_CH], F32)
            nc.sync.dma_start(out=b2_sb[:], in_=b2rep)
            iota_sb = cp.tile([P, P], BF16)
            nc.sync.dma_start(out=iota_sb[:], in_=iota)
            idf_sb = cp.tile([P, P], F32)
            nc.sync.dma_start(out=idf_sb[:], in_=identf)

            z_sb = cp.tile([P, NTILES, 64], BF16)     # z rows staged (64B pad)
            nc.vector.memset(z_sb[:], 0.0)            # zero pad cols once
            r_sb = cp.tile([P, NTILES, OUT_CH], F32)
            t_st = cp.tile([P, NTILES, OUT_CH], F32)
            nmax_st = cp.tile([P, NTILES], F32)
            sume_st = cp.tile([P, NTILES], F32)
            lse_st = cp.tile([P, NTILES], F32)
            out_st = cp.tile([P, NTILES, OUT_CH], F32)

            HS = SLOTS // 2                           # 3200 slots per piece
            z_myp = [dp.tile([HS, 64], BF16, tag=f"z_myp{k}", name=f"z_myp{k}")
                     for k in range(2)]
            z_allp = [dp.tile([NCORES * HS, 64], BF16, tag=f"z_allp{k}",
                              name=f"z_allp{k}") for k in range(2)]

            for _rep in range(reps):
                # ================= layer 1 =================
                for t in range(NTILES):
                    offA, nA, offB, nB, offc, nA16, nB16 = seg1[t]
                    cA, cB = nA // P, nB // P
                    c1 = cA + cB
                    m1 = wp.tile([P, c1max, P], BF16, tag="m1")
                    if no_g1:
                        # bisect: same bytes, contiguous rows instead of gather
                        nc.sync.dma_start(
                            out=m1[:, 0:c1, :],
                            in_=x_lo[(t % 25) * c1 * P:((t % 25) + 1) * c1 * P, :].rearrange(
                                "(c p) d -> p c d", p=P))
                    if nA and not no_g1:
                        nc.gpsimd.dma_gather(
                            out_ap=m1[:, 0:cA, :], in_ap=x_lo,
                            idxs_ap=idx1_sb[:, offA:offA + nA16 // 16],
                            num_idxs=nA16, num_idxs_reg=nA16, elem_size=P)
                    if nB and not no_g1:
                        nc.gpsimd.dma_gather(
                            out_ap=m1[:, cA:c1, :], in_ap=x_hi,
                            idxs_ap=idx1_sb[:, offB:offB + nB16 // 16],
                            num_idxs=nB16, num_idxs_reg=nB16, elem_size=P)
                    oh = wp.tile([P, c1max, P], BF16, tag="oh1")
                    nc.vector.tensor_tensor(
                        out=oh[:, 0:c1, :],
                        in0=dstv1_sb[:, offc:offc + c1].rearrange(
                            "p (c d) -> p c d", d=1).to_broadcast([P, c1, P]),
                        in1=iota_sb[:].rearrange("p (c d) -> p c d", c=1)
                            .to_broadcast([P, c1, P]),
                        op=OP.is_equal)
                    ps_mag = psm.tile([P, P], F32, space="PSUM", tag="psmag")
                    klims = _chunk_klims(nA, nB, nA16, nB16)
                    for c in (() if no_mm else range(c1)):
                        kl = klims[c]
                        nc.tensor.matmul(out=ps_mag[:], lhsT=m1[0:kl, c, :],
                                         rhs=oh[0:kl, c, :],
                                         start=(c == 0), stop=(c == c1 - 1))
                    mag_sb = wp.tile([P, P], BF16, tag="mag")
                    nc.vector.tensor_tensor(
                        out=mag_sb[:], in0=ps_mag[:],
                        in1=winv1_sb[:, t * P:(t + 1) * P], op=OP.mult)
                    ht = wp.tile([P, HB, P], BF16, tag="ht")
                    for j in (() if no_mm else range(HB)):
                        ps_h = psh.tile([P, P], F32, space="PSUM", tag="psh")
                        nc.tensor.matmul(out=ps_h[:], lhsT=wl1_sb[:, j * P:(j + 1) * P],
                                         rhs=mag_sb[:], start=True, stop=False)
                        nc.tensor.matmul(out=ps_h[:], lhsT=wr1_sb[:, j * P:(j + 1) * P],
                                         rhs=xT_sb[:, t * P:(t + 1) * P],
                                         start=False, stop=True)
                        nc.scalar.activation(out=ht[:, j, :], in_=ps_h[:],
                                             func=AF.Relu, bias=b1_sb[:, j:j + 1],
                                             scale=1.0)
                    ps_zr = psz.tile([2 * OUT_CH, P], F32, space="PSUM", tag="pszr")
                    for j in (() if no_mm else range(HB)):
                        nc.tensor.matmul(out=ps_zr[:], lhsT=w2_sb[:, j, :],
                                         rhs=ht[:, j, :],
                                         start=(j == 0), stop=(j == HB - 1))
                    zr_sb = wp.tile([2 * OUT_CH, P], F32, tag="zr")
                    nc.vector.tensor_copy(out=zr_sb[:], in_=ps_zr[:])
                    ps_t = pst_p.tile([P, 2 * OUT_CH], F32, space="PSUM", tag="pst")
                    nc.tensor.transpose(out=ps_t[:], in_=zr_sb[:],
                                        identity=idf_sb[0:2 * OUT_CH, 0:2 * OUT_CH])
                    nc.vector.tensor_copy(out=z_sb[:, t, 0:OUT_CH],
                                          in_=ps_t[:, 0:OUT_CH])
                    nc.vector.tensor_tensor(out=r_sb[:, t, :],
                                            in0=ps_t[:, OUT_CH:2 * OUT_CH],
                                            in1=b2_sb[:], op=OP.add)
                    # piece-wise z exchange overlapped with remaining L1 work
                    if not no_ag and t in (NTILES // 2 - 1, NTILES - 1):
                        k = 0 if t == NTILES // 2 - 1 else 1
                        ts0 = k * (NTILES // 2)
                        nc.sync.dma_start(
                            out=z_myp[k][:].rearrange("(t p) c -> p t c", p=P),
                            in_=z_sb[:, ts0:ts0 + NTILES // 2, :])
                        nc.gpsimd.collective_compute(
                            "AllGather", OP.bypass,
                            replica_groups=[list(range(NCORES))],
                            ins=[z_myp[k][:].opt()], outs=[z_allp[k][:].opt()])

                # ================= layer 2 =================
                # piece-0 gathers are prefetched LOOK tiles ahead so the Pool
                # engine has work while AG piece-1 is still completing
                LOOK = 2
                zg = [z_allp[k][:].rearrange("(a b) c -> a (b c)", b=2)
                      for k in range(2)]
                m2_tiles = {}

                def l2_prefetch_a(tt):
                    offA, nA, offB, nB, offc, nA16, nB16 = seg2[tt]
                    cA = nA // P
                    c2 = (nA + nB) // P
                    m2 = wp.tile([P, c2max, P], BF16, tag="m2", bufs=LOOK + 3)
                    m2_tiles[tt] = m2
                    if no_g2:
                        nc.sync.dma_start(
                            out=m2[:, 0:c2, :],
                            in_=zg[0][(tt % 25) * c2 * P:((tt % 25) + 1) * c2 * P, :]
                                .rearrange("(c p) d -> p c d", p=P))
                    if nA and not no_g2:
                        nc.gpsimd.dma_gather(
                            out_ap=m2[:, 0:cA, :], in_ap=zg[0],
                            idxs_ap=idx2_sb[:, offA:offA + nA16 // 16],
                            num_idxs=nA16, num_idxs_reg=nA16, elem_size=P)

                for tt in range(min(LOOK, NTILES)):
                    l2_prefetch_a(tt)
                for t in range(NTILES):
                    offA, nA, offB, nB, offc, nA16, nB16 = seg2[t]
                    cA, cB = nA // P, nB // P
                    c2 = cA + cB
                    m2 = m2_tiles.pop(t)
                    if nB and not no_g2:
                        nc.gpsimd.dma_gather(
                            out_ap=m2[:, cA:c2, :], in_ap=zg[1],
                            idxs_ap=idx2_sb[:, offB:offB + nB16 // 16],
                            num_idxs=nB16, num_idxs_reg=nB16, elem_size=P)
                    if t + LOOK < NTILES:
                        l2_prefetch_a(t + LOOK)
                    oh2e = wp.tile([P, c2max, P], BF16, tag="oh2e")
                    nc.vector.tensor_tensor(
                        out=oh2e[:, 0:c2, :],
                        in0=dstv2e_sb[:, offc:offc + c2].rearrange(
                            "p (c d) -> p c d", d=1).to_broadcast([P, c2, P]),
                        in1=iota_sb[:].rearrange("p (c d) -> p c d", c=1)
                            .to_broadcast([P, c2, P]),
                        op=OP.is_equal)
                    oh2o = wp.tile([P, c2max, P], BF16, tag="oh2o")
                    nc.vector.tensor_tensor(
                        out=oh2o[:, 0:c2, :],
                        in0=dstv2o_sb[:, offc:offc + c2].rearrange(
                            "p (c d) -> p c d", d=1).to_broadcast([P, c2, P]),
                        in1=iota_sb[:].rearrange("p (c d) -> p c d", c=1)
                            .to_broadcast([P, c2, P]),
                        op=OP.is_equal)
                    ps_o = pso_p.tile([P, 64], F32, space="PSUM", tag="pso")
                    klims = _chunk_klims(nA, nB, nA16, nB16)
                    for c in (() if no_mm else range(c2)):
                        kl = klims[c]
                        nc.tensor.matmul(out=ps_o[:], lhsT=oh2e[0:kl, c, :],
                                         rhs=m2[0:kl, c, 0:64],
                                         start=(c == 0), stop=False)
                        nc.tensor.matmul(out=ps_o[:], lhsT=oh2o[0:kl, c, :],
                                         rhs=m2[0:kl, c, 64:128],
                                         start=False, stop=(c == c2 - 1))
                    # t = agg*winv + (r+b2)
                    nc.vector.tensor_scalar(
                        out=t_st[:, t, :], in0=ps_o[:, 0:OUT_CH],
                        scalar1=winv2_sb[:, t:t + 1], scalar2=None, op0=OP.mult)
                    nc.vector.tensor_tensor(out=t_st[:, t, :], in0=t_st[:, t, :],
                                            in1=r_sb[:, t, :], op=OP.add)
                    rmax = wp.tile([P, 1], F32, tag="rmax")
                    nc.vector.tensor_reduce(out=rmax[:], in_=t_st[:, t, :],
                                            axis=mybir.AxisListType.X, op=OP.max)
                    nc.vector.tensor_scalar_mul(out=nmax_st[:, t:t + 1],
                                                in0=rmax[:], scalar1=-1.0)
                    e_sb = wp.tile([P, OUT_CH], F32, tag="esb")
                    nc.scalar.activation(out=e_sb[:], in_=t_st[:, t, :],
                                         func=AF.Exp, bias=nmax_st[:, t:t + 1],
                                         scale=1.0, accum_out=sume_st[:, t:t + 1])
                # ============ log-softmax finalize ============
                nc.scalar.activation(out=lse_st[:], in_=sume_st[:], func=AF.Ln)
                for t in range(NTILES):
                    nc.vector.tensor_scalar(
                        out=out_st[:, t, :], in0=t_st[:, t, :],
                        scalar1=nmax_st[:, t:t + 1], scalar2=lse_st[:, t:t + 1],
                        op0=OP.add, op1=OP.subtract)
                nc.sync.dma_start(
                    out=out.rearrange("(t p) c -> p t c", p=P), in_=out_st[:])
    nc.compile()
    return nc


# ---------------------------------------------------------------------------
# host-side preprocessing
# ---------------------------------------------------------------------------

def _wrap_idx(idx):
    """idx array (len multiple of 16) -> [128, len/16] int16 (k -> [k%16,
    k//16]), replicated to the 8 q7 groups."""
    n = len(idx)
    arr = np.zeros((16, n // 16), np.int16)
    arr[np.arange(n) % 16, np.arange(n) // 16] = idx
    return np.tile(arr, (8, 1))


def _prep(x, edge_index, Wl1, Wr1, b1, Wl2, Wr2, b2):
    src = edge_index[0].astype(np.int64)
    dst = edge_index[1].astype(np.int64)
    deg = np.bincount(dst, minlength=N_NODES)
    winv = 1.0 / np.maximum(deg, 1).astype(np.float32)

    # per-core slot assignment: deal nodes to tiles round-robin by degree
    slot_of = np.empty(N_NODES, np.int64)
    for c in range(NCORES):
        nids = np.arange(c * NPC, (c + 1) * NPC)
        order = nids[np.argsort(-deg[nids], kind="stable")]
        k = np.arange(NPC)
        slot_of[order] = (k % NTILES) * P + (k // NTILES)
    ecore = dst // NPC
    dslot = slot_of[dst]
    dtile = dslot // P
    dlane = dslot % P
    score = np.minimum(src // NPC, NCORES - 1)
    sslot = slot_of[src]
    HS = SLOTS // 2
    zpiece = sslot // HS                      # which AG piece holds src's z
    zrow = (score * HS + (sslot % HS)) // 2   # paired row in that piece
    zpar = sslot % 2                          # which 64-col half

    # collect per (core, tile) lo/hi edge lists for both layers
    ed1 = [[None] * NTILES for _ in range(NCORES)]
    ed2 = [[None] * NTILES for _ in range(NCORES)]
    for c in range(NCORES):
        sel = np.nonzero(ecore == c)[0]
        for t in range(NTILES):
            es = sel[dtile[sel] == t]
            lo = es[src[es] < XLO]
            hi = es[src[es] >= XLO]
            ed1[c][t] = (lo[np.argsort(src[lo], kind="stable")],
                         hi[np.argsort(src[hi], kind="stable")])
            lo = es[zpiece[es] == 0]
            hi = es[zpiece[es] == 1]
            ed2[c][t] = (lo[np.argsort(zrow[lo], kind="stable")],
                         hi[np.argsort(zrow[hi], kind="stable")])

    # unified segment sizes (max across cores, rounded to 128)
    def unify(ed):
        seg = []
        offw = offc = 0
        for t in range(NTILES):
            rA = max(len(ed[c][t][0]) for c in range(NCORES))
            rB = max(len(ed[c][t][1]) for c in range(NCORES))
            nA = ((rA + P - 1) // P) * P
            nB = ((rB + P - 1) // P) * P
            nA16 = ((rA + 15) // 16) * 16
            nB16 = ((rB + 15) // 16) * 16
            seg.append((offw, nA, offw + nA // 16, nB, offc, nA16, nB16))
            offw += (nA + nB) // 16
            offc += (nA + nB) // P
        return seg

    seg1 = unify(ed1)
    seg2 = unify(ed2)

    def pack(ed, seg, idx_of):
        """-> (idx [128, W] i16-wrapped, dstv [128, T])."""
        idxs, dvs = [], []
        for t in range(NTILES):
            _, nA, _, nB, _, _, _ = seg[t]
            lo, hi = ed[t]
            for es, n, rebase in ((lo, nA, 0), (hi, nB, 1)):
                iv = np.zeros(n, np.int16)
                dv = np.full(n, -1.0, np.float32)
                iv[:len(es)] = idx_of(es, rebase)
                dv[:len(es)] = dlane[es]
                idxs.append(iv)
                dvs.append(dv)
        idx = _wrap_idx(np.concatenate(idxs))
        dv = np.concatenate(dvs).reshape(-1, P).T.astype(ml_dtypes.bfloat16)
        return idx, np.ascontiguousarray(dv)

    def pack2(ed, seg):
        """L2: -> (idx paired-row i16, dstv_even, dstv_odd)."""
        idxs, dve, dvo = [], [], []
        for t in range(NTILES):
            _, nA, _, nB, _, _, _ = seg[t]
            for es, n in ((ed[t][0], nA), (ed[t][1], nB)):
                iv = np.zeros(n, np.int16)
                de = np.full(n, -1.0, np.float32)
                do = np.full(n, -1.0, np.float32)
                iv[:len(es)] = zrow[es].astype(np.int16)
                pe = zpar[es] == 0
                de[:len(es)][pe] = dlane[es][pe]
                do[:len(es)][~pe] = dlane[es][~pe]
                idxs.append(iv)
                dve.append(de)
                dvo.append(do)
        idx = _wrap_idx(np.concatenate(idxs))
        de = np.concatenate(dve).reshape(-1, P).T.astype(ml_dtypes.bfloat16)
        do = np.concatenate(dvo).reshape(-1, P).T.astype(ml_dtypes.bfloat16)
        return idx, np.ascontiguousarray(de), np.ascontiguousarray(do)

    x_lo = x[:XLO].astype(ml_dtypes.bfloat16)
    x_hi = x[XLO:].astype(ml_dtypes.bfloat16)
    iota = np.tile(np.arange(P, dtype=np.float32)[None, :], (P, 1))
    b1c = b1.reshape(HB, P).T.astype(np.float32).copy()
    W2h = np.ascontiguousarray(
        np.concatenate([Wl2, Wr2], axis=1).reshape(HB, P, 2 * OUT_CH)
        .transpose(1, 0, 2)).astype(ml_dtypes.bfloat16)

    in_maps = []
    for c in range(NCORES):
        i1, d1 = pack(ed1[c], seg1,
                      lambda es, hi: (src[es] - (XLO if hi else 0)).astype(np.int16))
        i2, d2e, d2o = pack2(ed2[c], seg2)
        nids = np.arange(c * NPC, (c + 1) * NPC)
        xs = np.zeros((SLOTS, IN_CH), np.float32)
        xs[slot_of[nids]] = x[nids]
        winv_slots = np.zeros(SLOTS, np.float32)
        winv_slots[slot_of[nids]] = winv[nids]
        in_maps.append({
            "x_lo": x_lo, "x_hi": x_hi,
            "xT": np.ascontiguousarray(xs.T).astype(ml_dtypes.bfloat16),
            "idx1": i1, "idx2": i2, "dstv1": d1,
            "dstv2e": d2e, "dstv2o": d2o,
            "winv1": np.broadcast_to(
                winv_slots.astype(ml_dtypes.bfloat16), (P, SLOTS)).copy(),
            "winv2": np.ascontiguousarray(
                winv_slots.reshape(NTILES, P).T).astype(np.float32),
            "Wl1": Wl1.astype(ml_dtypes.bfloat16),
            "Wr1": Wr1.astype(ml_dtypes.bfloat16),
            "W2": W2h, "b1c": b1c,
            "b2rep": np.broadcast_to(b2.astype(np.float32), (P, OUT_CH)).copy(),
            "iota": iota.astype(ml_dtypes.bfloat16),
            "identf": np.eye(P, dtype=np.float32),
        })
    return seg1, seg2, in_maps, slot_of


_cache = {}


def _get_nc(seg1, seg2, reps=1):
    key = (tuple(seg1), tuple(seg2), reps)
    if key not in _cache:
        _cache[key] = build_fused(seg1, seg2, reps=reps)
    return _cache[key]


def kernel(x, edge_index, Wl1, Wr1, b1, Wl2, Wr2, b2):
    x = np.asarray(x, np.float32)
    edge_index = np.asarray(edge_index)
    seg1, seg2, in_maps, slot_of = _prep(
        x, edge_index, np.asarray(Wl1, np.float32), np.asarray(Wr1, np.float32),
        np.asarray(b1, np.float32), np.asarray(Wl2, np.float32),
        np.asarray(Wr2, np.float32), np.asarray(b2, np.float32))
    nc = _get_nc(seg1, seg2, reps=1)
    res = bass_utils.run_bass_kernel_spmd(nc, in_maps, core_ids=list(range(NCORES)))
    out = np.empty((N_NODES, OUT_CH), np.float32)
    for c in range(NCORES):
        o = res.results[c]["out"]
        nids = np.arange(c * NPC, (c + 1) * NPC)
        out[nids] = o[slot_of[nids]]
    return out


# ---------------------------------------------------------------------------
# device-time measurement: launch-count slopes of 1-rep vs R-rep kernels.
# per-rep exec = (slope(R) - slope(1)) / (R - 1); tunnel/dispatch overhead
# cancels in the slope difference.
# ---------------------------------------------------------------------------

def _make_runner(nc, n_cores):
    import jax
    from jax.sharding import Mesh, PartitionSpec, NamedSharding
    from jax.experimental.shard_map import shard_map
    from concourse import bass2jax

    bass2jax.install_neuronx_cc_hook()
    pname = nc.partition_id_tensor.name if nc.partition_id_tensor else None
    in_names, out_names, out_avals = [], [], []
    for alloc in nc.m.functions[0].allocations:
        if not isinstance(alloc, mybir.MemoryLocationSet):
            continue
        name = alloc.memorylocations[0].name
        if alloc.kind == "ExternalInput":
            if name != pname:
                in_names.append(name)
        elif alloc.kind == "ExternalOutput":
            out_names.append(name)
            out_avals.append(jax.core.ShapedArray(
                tuple(alloc.tensor_shape), mybir.dt.np(alloc.dtype)))
    n_params = len(in_names)
    all_in = list(in_names) + list(out_names)
    if pname is not None:
        all_in.append(pname)

    def _body(*args):
        operands = list(args)
        if pname is not None:
            operands.append(bass2jax.partition_id_tensor())
        outs = bass2jax._bass_exec_p.bind(
            *operands, out_avals=tuple(out_avals), in_names=tuple(all_in),
            out_names=tuple(out_names), lowering_input_output_aliases=(),
            sim_require_finite=False, sim_require_nnan=False, nc=nc)
        return tuple(outs)

    devices = jax.devices()[:n_cores]
    mesh = Mesh(np.asarray(devices), ("core",))
    jitted = jax.jit(
        shard_map(_body, mesh=mesh,
                  in_specs=(PartitionSpec("core"),) * (n_params + len(out_names)),
                  out_specs=(PartitionSpec("core"),) * len(out_names),
                  check_rep=False),
        keep_unused=True)

    def prep(in_maps):
        concat = [np.concatenate([np.asarray(in_maps[c][n]) for c in range(n_cores)], 0)
                  for n in in_names]
        zeros = [np.zeros((n_cores * a.shape[0], *a.shape[1:]), a.dtype)
                 for a in out_avals]
        sh = NamedSharding(mesh, PartitionSpec("core"))
        return [jax.device_put(v, sh) for v in concat + zeros]

    return jitted, prep


def _slope(jitted, dev_in, ks=(1, 9, 17), sweeps=5):
    """Marginal wall time per pipelined call, via least squares over k."""
    import time as _time
    import jax
    jax.block_until_ready(jitted(*dev_in))
    samples = {k: [] for k in ks}
    for _ in range(sweeps):
        for k in ks:
            t0 = _time.perf_counter()
            for _i in range(k):
                o = jitted(*dev_in)
            jax.block_until_ready(o)
            samples[k].append(_time.perf_counter() - t0)
    xs = np.array(ks, np.float64)
    ys = np.array([np.median(samples[k]) for k in ks])
    A = np.stack([xs, np.ones_like(xs)], axis=1)
    slope, _ = np.linalg.lstsq(A, ys, rcond=None)[0]
    return float(slope)


def measure_exec_ns(inp, reps_hi=8):
    seg1, seg2, in_maps, _ = _prep(
        np.asarray(inp["x"], np.float32), np.asarray(inp["edge_index"]),
        np.asarray(inp["Wl1"], np.float32), np.asarray(inp["Wr1"], np.float32),
        np.asarray(inp["b1"], np.float32), np.asarray(inp["Wl2"], np.float32),
        np.asarray(inp["Wr2"], np.float32), np.asarray(inp["b2"], np.float32))
    nc1 = _get_nc(seg1, seg2, reps=1)
    ncR = _get_nc(seg1, seg2, reps=reps_hi)
    j1, prep1 = _make_runner(nc1, NCORES)
    d1 = prep1(in_maps)
    jR, prepR = _make_runner(ncR, NCORES)
    dR = prepR(in_maps)
    s1 = _slope(j1, d1)
    sR = _slope(jR, dR)
    per = (sR - s1) / (reps_hi - 1)
    print(f"  [slope-timing] slope(1)={s1*1e3:.3f} ms/call "
          f"slope({reps_hi})={sR*1e3:.3f} ms/call -> per-rep {per*1e6:.1f} us")
    return max(int(per * 1e9), 1)


build_fused_variant = build_fused
